# revision 24
# baseline (speedup 1.0000x reference)
"""Trainium2 Bass kernel for nn_ATTPool (attention-weighted temporal pooling).

Reference math (per batch b):
    att = (x_tre[b] + pos) @ W.T + bias              # (T=32, C=64)
    a   = softmax_T(att)                             # softmax over T
    out = sum_t a[t,c] * x[b,c,t,:,:] + x[b,c,T-1,:,:]   # (C, H*W)

pos-enc rows and the bias are constant along T (the softmax axis), so they
cancel exactly inside softmax: a = softmax_T(x_tre[b] @ W.T).  The
+x[:,:,-1] residual folds into the un-normalized weights as +sum_t exp at
t=T-1, and the 1/sum normalization folds into the epilogue PSUM->SBUF
copy as a per-partition scale.

Strategy: data-parallel over B=8 across the 8 NeuronCores.  Per core the
dominant work is streaming x[b] (32 MiB) once through TensorE: x[b] viewed
as (C*T=2048, HW=4096) is split into 16 chunks of 128 rows (4 channels x
32 timesteps).  A sparse (128, 64) weight tile routes row (c,t) to output
partition c with weight e[t,c]; matmuls accumulate into a persistent
(64, 4096) PSUM accumulator across all 16 chunks.

Precision/performance scheme: fp32 matmul costs ~5.5 cycles/column on
TRN2 (two half-rate passes), which made an fp32 version PE-bound at
~153 us vs the ~95 us DMA floor.  Instead x is split hi/lo into an fp16
pair on the host (same total bytes -> same DMA traffic) and the weights
into an fp16 pair on the device.  Each chunk then runs TWO fp16 passes:
    pass A: w_hi * x_hi
    pass Y: w_hi * y,   y = x_lo + g * x_hi,   g = w_lo / w_hi
so w_hi*y == w_hi*x_lo + w_lo*x_hi exactly; only the ~2^-22 w_lo*x_lo
term is dropped.  y is built per chunk on the otherwise-idle VectorE
(g is one scalar per (c,t) partition row).  Measured ~1e-5 max abs /
~9e-7 relative error vs the fp32 reference, and the kernel stays
DMA-bound (~360 GB/s) even when the chip's P0 power state drops the PE
from 2.4 to 2.0 GHz.

Hardware notes that shaped the code:
  * A Matmult can carry at most ONE semaphore wait pre-legalization;
    dependencies are funneled (single preamble DMA, two dummy matmuls
    absorbing the ACT/DVE waits) so hot-loop matmuls wait only on their
    x-chunk DMA or the chunk's y.
  * DVE cannot move data across partitions; the 4x partition-group
    replication of the attention rows is a tiny PE matmul against a
    stacked-identity selector (a DRAM round-trip costs ~20 us of
    serial latency by comparison).
  * DMA cannot touch PSUM, hence the ScalarE/VectorE copies in the
    epilogue (which also apply the 1/sum softmax normalization).
  * y = x_lo + g*x_hi is two DVE ops (tensor_scalar 4x mode + tensor
    _tensor 2x mode) rather than one scalar_tensor_tensor (1x only).
"""

import threading
from contextlib import ExitStack

import numpy as np

import concourse.bacc as bacc
import concourse.bass as bass
import concourse.tile as tile
from concourse import mybir
from concourse.bass_utils import run_bass_kernel_spmd

F32 = mybir.dt.float32
HALF = mybir.dt.float16      # hi/lo pair element type (11-bit mantissa)
NP_HALF = "float16"

B, C, T = 8, 64, 32
HW = 64 * 64                 # 4096
CT = C * T                   # 2048
NCHUNK = 16                  # chunks of 128 (c,t)-rows
NBANK = 8                    # 512-f32 matmul slices of the 4096 free dim
BANK = HW // NBANK           # 512
X_BUFS = 7                   # in-flight 1 MiB x half-chunk tiles per kind
Y_BUFS = 4                   # in-flight y correction tiles
# pre[] packing offsets (all fp32, 64 partitions)
PRE_WT, PRE_XT, PRE_ID, PRE_SEL = 0, 64, 96, 160
PRE_W = 288


def _build_nc() -> bass.Bass:
    nc = bacc.Bacc(None, target_bir_lowering=False)

    # x[b] split hi/lo: xhi = fp16(x), xlo = fp16(x - xhi)
    xhi = nc.dram_tensor("xhi", [CT, HW], HALF, kind="ExternalInput")
    xlo = nc.dram_tensor("xlo", [CT, HW], HALF, kind="ExternalInput")
    # pre = [W.T | x_tre.T | I64 | SEL] (transposes/constants built on host)
    pre = nc.dram_tensor("pre", [C, PRE_W], F32, kind="ExternalInput")
    out = nc.dram_tensor("out", [C, HW], F32, kind="ExternalOutput")

    with ExitStack() as ctx:
        tc = ctx.enter_context(tile.TileContext(nc))
        consts = ctx.enter_context(tc.tile_pool(name="consts", bufs=1))
        xpool = ctx.enter_context(tc.tile_pool(name="xp", bufs=X_BUFS))
        ypool = ctx.enter_context(tc.tile_pool(name="yp", bufs=Y_BUFS))
        psum = ctx.enter_context(
            tc.tile_pool(name="ps", bufs=1, space=bass.MemorySpace.PSUM)
        )

        # [0:64, :] is the pooled-output accumulator; bank-0..3 regions
        # double as preamble scratch (consumed before chunk 0's matmuls).
        acc = psum.tile([128, HW], F32)

        # weight tile zeroed first: no deps, fills the pre-DMA idle window
        lhsT_hi = consts.tile([128, NCHUNK * C], HALF)
        nc.vector.memset(lhsT_hi[:], 0.0)

        pre_sb = consts.tile([C, PRE_W], F32)
        nc.sync.dma_start(out=pre_sb[:], in_=pre[:])

        # --- attention preamble (tiny; overlaps chunk-0 x DMA) -----------
        # att^T[c, t] = sum_i W[c,i] * x_tre[t,i]        (bank 0 scratch)
        att_ps = acc[0:C, 0:T]
        nc.tensor.matmul(att_ps, pre_sb[:, PRE_WT:PRE_WT + C],
                         pre_sb[:, PRE_XT:PRE_XT + T])

        # e = exp(att) with fused row-sum; |att| <= ~7 so no max-subtract
        # is needed in fp32 (softmax is shift-invariant, matching ref).
        exp_s = consts.tile([C, T], F32)
        ssum = consts.tile([C, 1], F32)
        nc.scalar.activation(
            out=exp_s[:], in_=att_ps,
            func=mybir.ActivationFunctionType.Exp,
            bias=0.0, scale=1.0, accum_out=ssum[:],
        )
        # residual +x[:,:,T-1]: un-normalized weight +sum at t = T-1
        nc.vector.tensor_scalar_add(exp_s[:, T - 1 : T], exp_s[:, T - 1 : T], ssum[:])
        # epilogue scale 1/sum (off the critical path)
        rsum = consts.tile([C, 1], F32)
        nc.vector.reciprocal(rsum[:], ssum[:])

        # hi/lo fp16 split of the weights, packed [hi | lo] along free
        wpack = consts.tile([C, 2 * T], F32)
        hi_b = consts.tile([C, T], HALF)
        nc.vector.tensor_copy(out=hi_b[:], in_=exp_s[:])          # round to fp16
        nc.vector.tensor_copy(out=wpack[:, 0:T], in_=hi_b[:])     # back to f32
        nc.vector.tensor_sub(wpack[:, T : 2 * T], exp_s[:], wpack[:, 0:T])

        # PE transpose halves: tr_H[t, c] = w_H[c, t]    (bank 0/1 scratch)
        ident = pre_sb[:, PRE_ID:PRE_ID + C]
        tr_ps = [acc[0:T, 512 * H + 256 : 512 * H + 320] for H in (0, 1)]
        nc.tensor.transpose(tr_ps[0], wpack[:, 0:T], ident)
        nc.tensor.transpose(tr_ps[1], wpack[:, T : 2 * T], ident)
        tr_sb = consts.tile([T, 2 * C], F32)
        nc.vector.tensor_copy(out=tr_sb[:, 0:C], in_=tr_ps[0])
        nc.vector.tensor_copy(out=tr_sb[:, C : 2 * C], in_=tr_ps[1])

        # Cross-partition replicate: rep_H[32r+t, c] = tr_H[t, c] via a
        # matmul against SEL = [I32 I32 I32 I32]      (bank 2/3 scratch)
        sel = pre_sb[0:T, PRE_SEL:PRE_SEL + 128]
        rep_ps = []
        for H in range(2):
            rp = acc[0:128, 1024 + 512 * H : 1024 + 512 * H + C]
            rep_ps.append(rp)
            nc.tensor.matmul(rp, sel, tr_sb[0:T, C * H : C * (H + 1)])

        # Sparse routing weights for all 16 chunks (fp16): chunk k is
        # lhsT[:, 64k:64k+64]; its column c = 4k+j is nonzero on
        # partitions [32j, 32j+32) with values w_hi[t, 4k+j].  Writing
        # rep[32j+t, j+4m] -> lhsT[32j+t, j+68m] places exactly those.
        for j in range(4):
            nc.vector.tensor_copy(
                out=lhsT_hi[32 * j : 32 * (j + 1), j :: 68],
                in_=rep_ps[0][32 * j : 32 * (j + 1), j : C : 4],
            )
        # g[(c,t)] = w_lo/w_hi: per-partition scalars for the y pass;
        # g_all[:, k] holds chunk k's 128 row factors
        grec = consts.tile([128, C], F32)
        nc.vector.reciprocal(grec[:], rep_ps[0])
        gq = consts.tile([128, C], F32)
        nc.vector.tensor_mul(gq[:], grec[:], rep_ps[1])
        g_all = consts.tile([128, NCHUNK], F32)
        for j in range(4):
            nc.vector.tensor_copy(
                out=g_all[32 * j : 32 * (j + 1), :],
                in_=gq[32 * j : 32 * (j + 1), j : C : 4],
            )
        # chunk 15 runs the direct 3-pass form (no y) so no VectorE work
        # gates the tail; it needs w_lo for its 4 channels
        lhsT_lo15 = consts.tile([128, C], HALF)
        nc.vector.memset(lhsT_lo15[:], 0.0)
        for j in range(4):
            cc = C - 4 + j
            nc.vector.tensor_copy(
                out=lhsT_lo15[32 * j : 32 * (j + 1), cc : cc + 1],
                in_=rep_ps[1][32 * j : 32 * (j + 1), cc : cc + 1],
            )

        # Two dummy matmuls funnel the remaining preamble waits (ACT/DVE)
        # so hot-loop matmuls each carry a single wait.
        nc.tensor.matmul(acc[0:4, 0:4], pre_sb[:, 0:4], pre_sb[:, 0:4])
        nc.tensor.matmul(acc[0:4, 0:4], lhsT_hi[:, 0:4], lhsT_lo15[:, 60:64])

        # --- main loop: stream x[b] through TensorE ----------------------
        for k in range(NCHUNK):
            rows = slice(128 * k, 128 * (k + 1))
            g_k = g_all[:, k : k + 1]
            w_hi = lhsT_hi[:, C * k : C * (k + 1)]
            thi = xpool.tile([128, HW], HALF, tag="xhi")
            tlo = xpool.tile([128, HW], HALF, tag="xlo")
            y = ypool.tile([128, HW], HALF, tag="y")
            ring_a = nc.sync if k % 2 == 0 else nc.scalar
            ring_b = nc.scalar if k % 2 == 0 else nc.sync
            if k < NCHUNK - 1:
                # two HWDGE rings, each carrying half of both streams:
                # more outstanding packets and self-balancing against
                # per-ring jitter
                ring_a.dma_start(out=thi[:], in_=xhi[rows, :])
                ring_b.dma_start(out=tlo[:], in_=xlo[rows, :])
                # y = x_lo + g*x_hi on the otherwise-idle VectorE
                nc.vector.tensor_scalar_mul(y[:], thi[:], g_k)
                nc.vector.tensor_add(y[:], y[:], tlo[:])
                for xt, first, last in ((thi, k == 0, False), (y, False, False)):
                    for n in range(NBANK):
                        nc.tensor.matmul(
                            acc[0:C, BANK * n : BANK * (n + 1)],
                            w_hi,
                            xt[:, BANK * n : BANK * (n + 1)],
                            start=first,
                            stop=last,
                        )
            else:
                # last chunk: quarter-split the DMAs, y, matmuls, and the
                # epilogue pairs so everything drains as data lands
                for q in range(4):
                    cols = slice(1024 * q, 1024 * (q + 1))
                    ring_a.dma_start(out=thi[:, cols], in_=xhi[rows, cols])
                    ring_b.dma_start(out=tlo[:, cols], in_=xlo[rows, cols])
                for q in range(4):
                    for n in (2 * q, 2 * q + 1):
                        bcols = slice(BANK * n, BANK * (n + 1))
                        nc.tensor.matmul(acc[0:C, bcols], w_hi, thi[:, bcols],
                                         start=False, stop=False)
                        nc.tensor.matmul(acc[0:C, bcols], w_hi, tlo[:, bcols],
                                         start=False, stop=False)
                        nc.tensor.matmul(acc[0:C, bcols], lhsT_lo15[:],
                                         thi[:, bcols], start=False, stop=True)

        # --- epilogue: per-bank (PSUM * 1/sum) -> SBUF -> HBM ------------
        # split across ScalarE and VectorE so the copies keep pace with
        # the staggered bank completions
        out_sb = consts.tile([C, HW], F32)
        for n in range(NBANK):
            dst = out_sb[:, BANK * n : BANK * (n + 1)]
            src = acc[0:C, BANK * n : BANK * (n + 1)]
            if n % 2 == 0:
                nc.scalar.activation(
                    out=dst, in_=src,
                    func=mybir.ActivationFunctionType.Copy,
                    bias=0.0, scale=rsum[:],
                )
            else:
                nc.vector.tensor_scalar_mul(dst, src, rsum[:])
            (nc.sync if n % 2 else nc.scalar).dma_start(
                out=out[:, BANK * n : BANK * (n + 1)],
                in_=out_sb[:, BANK * n : BANK * (n + 1)],
            )

    nc.compile()
    return nc


_NC_LOCK = threading.Lock()
_NC_CACHE: list = []


def _get_nc() -> bass.Bass:
    with _NC_LOCK:
        if not _NC_CACHE:
            _NC_CACHE.append(_build_nc())
        return _NC_CACHE[0]


def _make_pre(WT: np.ndarray, x_tre_b: np.ndarray) -> np.ndarray:
    pre = np.zeros((C, PRE_W), dtype=np.float32)
    pre[:, PRE_WT:PRE_WT + C] = WT
    pre[:, PRE_XT:PRE_XT + T] = x_tre_b.T
    pre[:, PRE_ID:PRE_ID + C] = np.eye(C, dtype=np.float32)
    for r in range(4):
        pre[0:T, PRE_SEL + T * r : PRE_SEL + T * (r + 1)] = np.eye(
            T, dtype=np.float32
        )
    return pre


def run(x, x_tre, W, b=None, trace: bool = False, trace_cores=None):
    """Run the SPMD kernel on 8 cores; returns (BassKernelResults, output)."""
    x = np.asarray(x, dtype=np.float32)
    x_tre = np.asarray(x_tre, dtype=np.float32)
    WT = np.ascontiguousarray(np.asarray(W, dtype=np.float32).T)
    maps = []
    for core in range(B):
        xc = x[core].reshape(CT, HW)
        hi = xc.astype(NP_HALF)
        lo = (xc - hi.astype(np.float32)).astype(NP_HALF)
        maps.append(
            {
                "xhi": np.ascontiguousarray(hi),
                "xlo": np.ascontiguousarray(lo),
                "pre": _make_pre(WT, np.asarray(x_tre[core], np.float32)),
            }
        )
    nc = _get_nc()
    kw = {"trace_cores": trace_cores} if trace_cores else {}
    res = run_bass_kernel_spmd(nc, maps, core_ids=list(range(B)), trace=trace, **kw)
    outs = np.stack([np.asarray(r["out"]).reshape(C, 64, 64) for r in res.results])
    return res, outs.astype(np.float32)


def kernel(x, x_tre, W, b=None, **_unused):
    _, out = run(x, x_tre, W, b)
    return out


# revision 25
# speedup vs baseline: 1.0495x; 1.0495x over previous
"""Trainium2 Bass kernel for nn_ATTPool (attention-weighted temporal pooling).

Reference math (per batch b):
    att = (x_tre[b] + pos) @ W.T + bias              # (T=32, C=64)
    a   = softmax_T(att)                             # softmax over T
    out = sum_t a[t,c] * x[b,c,t,:,:] + x[b,c,T-1,:,:]   # (C, H*W)

pos-enc rows and the bias are constant along T (the softmax axis), so they
cancel exactly inside softmax: a = softmax_T(x_tre[b] @ W.T).  The
+x[:,:,-1] residual folds into the un-normalized weights as +sum_t exp at
t=T-1, and the 1/sum normalization folds into the epilogue PSUM->SBUF
copy as a per-partition scale.

Strategy: data-parallel over B=8 across the 8 NeuronCores.  Per core the
dominant work is streaming x[b] (32 MiB) once through TensorE: x[b] viewed
as (C*T=2048, HW=4096) is split into 16 chunks of 128 rows (4 channels x
32 timesteps).  A sparse (128, 64) weight tile routes row (c,t) to output
partition c with weight e[t,c]; matmuls accumulate into a persistent
(64, 4096) PSUM accumulator across all 16 chunks.

Precision/performance scheme: fp32 matmul costs ~5.5 cycles/column on
TRN2 (two half-rate passes), which made an fp32 version PE-bound at
~153 us vs the ~95 us DMA floor.  Instead x is split hi/lo into an fp16
pair on the host (same total bytes -> same DMA traffic) and the weights
into an fp16 pair on the device.  Each chunk then runs TWO fp16 passes:
    pass A: w_hi * x_hi
    pass Y: w_hi * y,   y = x_lo + g * x_hi,   g = w_lo / w_hi
so w_hi*y == w_hi*x_lo + w_lo*x_hi exactly; only the ~2^-22 w_lo*x_lo
term is dropped.  y is built per chunk on the otherwise-idle VectorE
(g is one scalar per (c,t) partition row).  Measured ~1e-5 max abs /
~9e-7 relative error vs the fp32 reference, and the kernel stays
DMA-bound (~360 GB/s) even when the chip's P0 power state drops the PE
from 2.4 to 2.0 GHz.

Hardware notes that shaped the code:
  * A Matmult can carry at most ONE semaphore wait pre-legalization;
    dependencies are funneled (single preamble DMA, two dummy matmuls
    absorbing the ACT/DVE waits) so hot-loop matmuls wait only on their
    x-chunk DMA or the chunk's y.
  * DVE cannot move data across partitions; the 4x partition-group
    replication of the attention rows is a tiny PE matmul against a
    stacked-identity selector (a DRAM round-trip costs ~20 us of
    serial latency by comparison).
  * DMA cannot touch PSUM, hence the ScalarE/VectorE copies in the
    epilogue (which also apply the 1/sum softmax normalization).
  * y = x_lo + g*x_hi is two DVE ops (tensor_scalar 4x mode + tensor
    _tensor 2x mode) rather than one scalar_tensor_tensor (1x only).
"""

import threading
from contextlib import ExitStack

import numpy as np

import concourse.bacc as bacc
import concourse.bass as bass
import concourse.tile as tile
from concourse import mybir
from concourse.bass_utils import run_bass_kernel_spmd

F32 = mybir.dt.float32
HALF = mybir.dt.float16      # hi/lo pair element type (11-bit mantissa)
NP_HALF = "float16"

B, C, T = 8, 64, 32
HW = 64 * 64                 # 4096
CT = C * T                   # 2048
NCHUNK = 16                  # chunks of 128 (c,t)-rows
NBANK = 8                    # 512-f32 matmul slices of the 4096 free dim
BANK = HW // NBANK           # 512
X_BUFS = 7                   # in-flight 1 MiB x half-chunk tiles per kind
Y_BUFS = 4                   # in-flight y correction tiles
# pre[] packing offsets (all fp32, 64 partitions)
PRE_WT, PRE_XT, PRE_ID, PRE_SEL = 0, 64, 96, 160
PRE_W = 288


def _build_nc() -> bass.Bass:
    nc = bacc.Bacc(None, target_bir_lowering=False)

    # x[b] split hi/lo: xhi = fp16(x), xlo = fp16(x - xhi)
    xhi = nc.dram_tensor("xhi", [CT, HW], HALF, kind="ExternalInput")
    xlo = nc.dram_tensor("xlo", [CT, HW], HALF, kind="ExternalInput")
    # pre = [W.T | x_tre.T | I64 | SEL] (transposes/constants built on host)
    pre = nc.dram_tensor("pre", [C, PRE_W], F32, kind="ExternalInput")
    out = nc.dram_tensor("out", [C, HW], F32, kind="ExternalOutput")

    with ExitStack() as ctx:
        tc = ctx.enter_context(tile.TileContext(nc))
        consts = ctx.enter_context(tc.tile_pool(name="consts", bufs=1))
        xpool = ctx.enter_context(tc.tile_pool(name="xp", bufs=X_BUFS))
        ypool = ctx.enter_context(tc.tile_pool(name="yp", bufs=Y_BUFS))
        psum = ctx.enter_context(
            tc.tile_pool(name="ps", bufs=1, space=bass.MemorySpace.PSUM)
        )

        # [0:64, :] is the pooled-output accumulator; bank-0..3 regions
        # double as preamble scratch (consumed before chunk 0's matmuls).
        acc = psum.tile([128, HW], F32)

        # weight tile zeroed first: no deps, fills the pre-DMA idle window
        lhsT_hi = consts.tile([128, NCHUNK * C], HALF)
        nc.vector.memset(lhsT_hi[:], 0.0)

        pre_sb = consts.tile([C, PRE_W], F32)
        nc.sync.dma_start(out=pre_sb[:], in_=pre[:])

        # --- attention preamble (tiny; overlaps chunk-0 x DMA) -----------
        # att^T[c, t] = sum_i W[c,i] * x_tre[t,i]        (bank 0 scratch)
        att_ps = acc[0:C, 0:T]
        nc.tensor.matmul(att_ps, pre_sb[:, PRE_WT:PRE_WT + C],
                         pre_sb[:, PRE_XT:PRE_XT + T])

        # e = exp(att) with fused row-sum; |att| <= ~7 so no max-subtract
        # is needed in fp32 (softmax is shift-invariant, matching ref).
        exp_s = consts.tile([C, T], F32)
        ssum = consts.tile([C, 1], F32)
        nc.scalar.activation(
            out=exp_s[:], in_=att_ps,
            func=mybir.ActivationFunctionType.Exp,
            bias=0.0, scale=1.0, accum_out=ssum[:],
        )
        # residual +x[:,:,T-1]: un-normalized weight +sum at t = T-1
        nc.vector.tensor_scalar_add(exp_s[:, T - 1 : T], exp_s[:, T - 1 : T], ssum[:])
        # epilogue scale 1/sum (off the critical path)
        rsum = consts.tile([C, 1], F32)
        nc.vector.reciprocal(rsum[:], ssum[:])

        # hi/lo fp16 split of the weights, packed [hi | lo] along free
        wpack = consts.tile([C, 2 * T], F32)
        hi_b = consts.tile([C, T], HALF)
        nc.vector.tensor_copy(out=hi_b[:], in_=exp_s[:])          # round to fp16
        nc.vector.tensor_copy(out=wpack[:, 0:T], in_=hi_b[:])     # back to f32
        nc.vector.tensor_sub(wpack[:, T : 2 * T], exp_s[:], wpack[:, 0:T])

        # PE transpose halves: tr_H[t, c] = w_H[c, t]    (bank 0/1 scratch)
        ident = pre_sb[:, PRE_ID:PRE_ID + C]
        tr_ps = [acc[0:T, 512 * H + 256 : 512 * H + 320] for H in (0, 1)]
        nc.tensor.transpose(tr_ps[0], wpack[:, 0:T], ident)
        nc.tensor.transpose(tr_ps[1], wpack[:, T : 2 * T], ident)
        tr_sb = consts.tile([T, 2 * C], F32)
        nc.vector.tensor_copy(out=tr_sb[:, 0:C], in_=tr_ps[0])
        nc.vector.tensor_copy(out=tr_sb[:, C : 2 * C], in_=tr_ps[1])

        # Cross-partition replicate: rep_H[32r+t, c] = tr_H[t, c] via a
        # matmul against SEL = [I32 I32 I32 I32]      (bank 2/3 scratch)
        sel = pre_sb[0:T, PRE_SEL:PRE_SEL + 128]
        rep_ps = []
        for H in range(2):
            rp = acc[0:128, 1024 + 512 * H : 1024 + 512 * H + C]
            rep_ps.append(rp)
            nc.tensor.matmul(rp, sel, tr_sb[0:T, C * H : C * (H + 1)])

        # Sparse routing weights for all 16 chunks (fp16): chunk k is
        # lhsT[:, 64k:64k+64]; its column c = 4k+j is nonzero on
        # partitions [32j, 32j+32) with values w_hi[t, 4k+j].  Writing
        # rep[32j+t, j+4m] -> lhsT[32j+t, j+68m] places exactly those.
        for j in range(4):
            nc.vector.tensor_copy(
                out=lhsT_hi[32 * j : 32 * (j + 1), j :: 68],
                in_=rep_ps[0][32 * j : 32 * (j + 1), j : C : 4],
            )
        # g[(c,t)] = w_lo/w_hi: per-partition scalars for the y pass;
        # g_all[:, k] holds chunk k's 128 row factors
        grec = consts.tile([128, C], F32)
        nc.vector.reciprocal(grec[:], rep_ps[0])
        gq = consts.tile([128, C], F32)
        nc.vector.tensor_mul(gq[:], grec[:], rep_ps[1])
        g_all = consts.tile([128, NCHUNK], F32)
        for j in range(4):
            nc.vector.tensor_copy(
                out=g_all[32 * j : 32 * (j + 1), :],
                in_=gq[32 * j : 32 * (j + 1), j : C : 4],
            )
        # the last two chunks run the direct 3-pass form (no y) so no
        # VectorE work gates the tail; each needs w_lo for its 4 channels
        lhsT_lo_tail = {}
        for kk in (NCHUNK - 2, NCHUNK - 1):
            wl = consts.tile([128, C], HALF, tag=f"wlo{kk}")
            lhsT_lo_tail[kk] = wl
            nc.vector.memset(wl[:], 0.0)
            for j in range(4):
                cc = 4 * kk + j
                nc.vector.tensor_copy(
                    out=wl[32 * j : 32 * (j + 1), cc : cc + 1],
                    in_=rep_ps[1][32 * j : 32 * (j + 1), cc : cc + 1],
                )

        # Two dummy matmuls funnel the remaining preamble waits (ACT/DVE)
        # so hot-loop matmuls each carry a single wait.
        nc.tensor.matmul(acc[0:4, 0:4], pre_sb[:, 0:4], pre_sb[:, 0:4])
        nc.tensor.matmul(acc[0:4, 0:4], lhsT_hi[:, 0:4],
                 lhsT_lo_tail[NCHUNK - 1][:, C - 4 : C])

        # --- main loop: stream x[b] through TensorE ----------------------
        for k in range(NCHUNK):
            rows = slice(128 * k, 128 * (k + 1))
            g_k = g_all[:, k : k + 1]
            w_hi = lhsT_hi[:, C * k : C * (k + 1)]
            thi = xpool.tile([128, HW], HALF, tag="xhi")
            tlo = xpool.tile([128, HW], HALF, tag="xlo")
            ring_a = nc.sync if k % 2 == 0 else nc.scalar
            ring_b = nc.scalar if k % 2 == 0 else nc.sync
            if k < NCHUNK - 2:
                # two HWDGE rings, each carrying half of both streams:
                # more outstanding packets and self-balancing against
                # per-ring jitter
                ring_a.dma_start(out=thi[:], in_=xhi[rows, :])
                ring_b.dma_start(out=tlo[:], in_=xlo[rows, :])
                # y = x_lo + g*x_hi on the otherwise-idle VectorE
                y = ypool.tile([128, HW], HALF, tag="y")
                nc.vector.tensor_scalar_mul(y[:], thi[:], g_k)
                nc.vector.tensor_add(y[:], y[:], tlo[:])
                for xt, first, last in ((thi, k == 0, False), (y, False, False)):
                    for n in range(NBANK):
                        nc.tensor.matmul(
                            acc[0:C, BANK * n : BANK * (n + 1)],
                            w_hi,
                            xt[:, BANK * n : BANK * (n + 1)],
                            start=first,
                            stop=last,
                        )
            else:
                # last chunk: quarter-split the DMAs, y, matmuls, and the
                # epilogue pairs so everything drains as data lands
                for q in range(4):
                    cols = slice(1024 * q, 1024 * (q + 1))
                    ring_a.dma_start(out=thi[:, cols], in_=xhi[rows, cols])
                    ring_b.dma_start(out=tlo[:, cols], in_=xlo[rows, cols])
                w_lo = lhsT_lo_tail[k]
                last = k == NCHUNK - 1
                for q in range(4):
                    for n in (2 * q, 2 * q + 1):
                        bcols = slice(BANK * n, BANK * (n + 1))
                        nc.tensor.matmul(acc[0:C, bcols], w_hi, thi[:, bcols],
                                         start=False, stop=False)
                        nc.tensor.matmul(acc[0:C, bcols], w_hi, tlo[:, bcols],
                                         start=False, stop=False)
                        nc.tensor.matmul(acc[0:C, bcols], w_lo[:],
                                         thi[:, bcols], start=False, stop=last)

        # --- epilogue: per-bank (PSUM * 1/sum) -> SBUF -> HBM ------------
        # split across ScalarE and VectorE so the copies keep pace with
        # the staggered bank completions
        out_sb = consts.tile([C, HW], F32)
        for n in range(NBANK):
            dst = out_sb[:, BANK * n : BANK * (n + 1)]
            src = acc[0:C, BANK * n : BANK * (n + 1)]
            if n % 2 == 0:
                nc.scalar.activation(
                    out=dst, in_=src,
                    func=mybir.ActivationFunctionType.Copy,
                    bias=0.0, scale=rsum[:],
                )
            else:
                nc.vector.tensor_scalar_mul(dst, src, rsum[:])
            (nc.sync if n % 2 else nc.scalar).dma_start(
                out=out[:, BANK * n : BANK * (n + 1)],
                in_=out_sb[:, BANK * n : BANK * (n + 1)],
            )

    nc.compile()
    return nc


_NC_LOCK = threading.Lock()
_NC_CACHE: list = []


def _get_nc() -> bass.Bass:
    with _NC_LOCK:
        if not _NC_CACHE:
            _NC_CACHE.append(_build_nc())
        return _NC_CACHE[0]


def _make_pre(WT: np.ndarray, x_tre_b: np.ndarray) -> np.ndarray:
    pre = np.zeros((C, PRE_W), dtype=np.float32)
    pre[:, PRE_WT:PRE_WT + C] = WT
    pre[:, PRE_XT:PRE_XT + T] = x_tre_b.T
    pre[:, PRE_ID:PRE_ID + C] = np.eye(C, dtype=np.float32)
    for r in range(4):
        pre[0:T, PRE_SEL + T * r : PRE_SEL + T * (r + 1)] = np.eye(
            T, dtype=np.float32
        )
    return pre


def run(x, x_tre, W, b=None, trace: bool = False, trace_cores=None):
    """Run the SPMD kernel on 8 cores; returns (BassKernelResults, output)."""
    x = np.asarray(x, dtype=np.float32)
    x_tre = np.asarray(x_tre, dtype=np.float32)
    WT = np.ascontiguousarray(np.asarray(W, dtype=np.float32).T)
    maps = []
    for core in range(B):
        xc = x[core].reshape(CT, HW)
        hi = xc.astype(NP_HALF)
        lo = (xc - hi.astype(np.float32)).astype(NP_HALF)
        maps.append(
            {
                "xhi": np.ascontiguousarray(hi),
                "xlo": np.ascontiguousarray(lo),
                "pre": _make_pre(WT, np.asarray(x_tre[core], np.float32)),
            }
        )
    nc = _get_nc()
    kw = {"trace_cores": trace_cores} if trace_cores else {}
    res = run_bass_kernel_spmd(nc, maps, core_ids=list(range(B)), trace=trace, **kw)
    outs = np.stack([np.asarray(r["out"]).reshape(C, 64, 64) for r in res.results])
    return res, outs.astype(np.float32)


def kernel(x, x_tre, W, b=None, **_unused):
    _, out = run(x, x_tre, W, b)
    return out
